# revision 4
# baseline (speedup 1.0000x reference)
"""FiLM-with-attention Trainium2 kernel.

Full inputs in, full output out. Sharding: data-parallel over batch B=8
across the 8 NeuronCores (one batch element per core); weights replicated.

Per-core pipeline (Lx = Lz = 2048, D = 256):
  - host pre-transposes x, z to feature-major fp16 and pre-scales Wq/bq by
    1/sqrt(D)
  - qT = Wq^T @ xT, kT = Wk^T @ zT   (feature-major, fp16)
  - v  = z @ Wv + bv                 (bias folded in as a K=1 matmul)
  - scores^T[j, i] = kT^T qT         (j on partitions -> z-mask is a
                                      per-partition bias on the Exp
                                      activation; exp(-30000) == 0)
  - S[i] = sum_j E[j, i]             (ones-vector matmul column sum)
  - ctxT[d, i] = sum_j v[j, d] E[j, i]
  - gb = ctxT^T @ Wo + S * bo        (bo folded as rank-1 S x bo matmul so
                                      the 1/S normalization can be applied
                                      once at the end)
  - out = (gb[:, :D] * x + gb[:, D:]) / S
"""

import sys

import numpy as np

sys.path.insert(0, "/opt/trn_rl_repo")

import concourse.bass as bass
import concourse.mybir as mybir
import concourse.tile as tile
from concourse.bass_utils import run_bass_kernel_spmd
from concourse.vector_clock import ScopedClock

# The walrus build in this container encodes at most ONE sync-wait command
# per instruction ("Too many sync wait commands" otherwise), while Tile's
# sem-assignment attaches one wait per dependee. Split: extra waits move to
# single-wait NoOp instructions on the same engine (sem-ge waits commute, so
# a chain of single-wait stalls is equivalent to one multi-wait stall).
_orig_add_instruction = tile.TileContext._add_instruction
_orig_drain_and_barrier = tile.TileContext._drain_and_barrier


def _split_add_instruction(self, inst):
    si = getattr(inst, "sync_info", None)
    if si is not None and si.on_wait is not None and len(si.on_wait) > 1:
        waits = list(si.on_wait)
        for w in waits[:-1]:
            nop = mybir.InstNoOp(
                name=self.nc.get_next_instruction_name(), ins=[], outs=[]
            )
            nop.engine = inst.engine
            nop.sync_info = mybir.SyncInfo(on_wait=[w], on_update=[])
            _orig_add_instruction(self, nop)
        si.on_wait = [waits[-1]]
    _orig_add_instruction(self, inst)


def _split_drain_and_barrier(self, tick_clock, wait_clock):
    nc = self.nc
    drain_inst = nc.sync.drain()
    wait_clock.add_sem_waits(
        drain_inst.ins, ScopedClock({None: tick_clock.global_clock})
    )
    si = drain_inst.ins.sync_info
    if si is not None and si.on_wait is not None and len(si.on_wait) > 1:
        waits = list(si.on_wait)
        si.on_wait = [waits[0]]
        for w in waits[1:]:
            nop_bi = nc.sync.nop()
            nop_bi.ins.sync_info = mybir.SyncInfo(on_wait=[w], on_update=[])

    nc.all_engine_barrier()
    assert self.sems is not None
    popped = nc._tile_sem_poison_stack.pop()
    assert popped is self._sem_poison
    nc.clear_and_free_semaphores(list(self.sems.allocated().values()))
    nc.all_engine_barrier()


tile.TileContext._add_instruction = _split_add_instruction
tile.TileContext._drain_and_barrier = _split_drain_and_barrier

B, L, D = 8, 2048, 256
D2 = 2 * D
P = 128
NCORES = 8
NKT = D // P  # k-tiles over the feature dim (2)
NJT = L // P  # j-tiles over Lz (16)
NIC = 4  # i-chunks over Lx
IC = L // NIC  # 512
NIT = IC // P  # i-subtiles per chunk (4)
FP16 = mybir.dt.float16
F32 = mybir.dt.float32
NEG = -30000.0  # exp(x + NEG) == 0.0 for any realistic score x

TRACE = False
LAST_RESULTS = None

_cache = {}


def _build_nc():
    nc = bass.Bass("TRN2", target_bir_lowering=False, debug=False)

    xT = nc.declare_dram_parameter("xT", [D, L], FP16, isOutput=False)
    zT = nc.declare_dram_parameter("zT", [D, L], FP16, isOutput=False)
    xf = nc.declare_dram_parameter("xf", [L, D], F32, isOutput=False)
    wq = nc.declare_dram_parameter("wq", [D, D], FP16, isOutput=False)
    wk = nc.declare_dram_parameter("wk", [D, D], FP16, isOutput=False)
    wv = nc.declare_dram_parameter("wv", [D, D], FP16, isOutput=False)
    wo = nc.declare_dram_parameter("wo", [D, D2], FP16, isOutput=False)
    bq = nc.declare_dram_parameter("bq", [P, NKT], F32, isOutput=False)
    bk = nc.declare_dram_parameter("bk", [P, NKT], F32, isOutput=False)
    bv = nc.declare_dram_parameter("bv", [1, D], FP16, isOutput=False)
    bo = nc.declare_dram_parameter("bo", [1, D2], FP16, isOutput=False)
    zb = nc.declare_dram_parameter("zb", [P, NJT], F32, isOutput=False)
    out = nc.declare_dram_parameter("out", [L, D], F32, isOutput=True)

    ADD = mybir.AluOpType.add
    MUL = mybir.AluOpType.mult
    EXP = mybir.ActivationFunctionType.Exp

    with tile.TileContext(nc) as tc:
        with (
            tc.tile_pool(name="consts", bufs=1) as consts,
            tc.tile_pool(name="data", bufs=1) as data,
        ):
            wq_sb = consts.tile([P, NKT, D], FP16)
            nc.sync.dma_start(wq_sb[:], wq.ap().rearrange("(ko p) m -> p ko m", p=P))
            wk_sb = consts.tile([P, NKT, D], FP16)
            nc.sync.dma_start(wk_sb[:], wk.ap().rearrange("(ko p) m -> p ko m", p=P))
            wv_sb = consts.tile([P, NKT, D], FP16)
            nc.sync.dma_start(wv_sb[:], wv.ap().rearrange("(ko p) m -> p ko m", p=P))
            wo_sb = consts.tile([P, NKT, D2], FP16)
            nc.sync.dma_start(wo_sb[:], wo.ap().rearrange("(ko p) m -> p ko m", p=P))
            bq_sb = consts.tile([P, NKT], F32)
            nc.sync.dma_start(bq_sb[:], bq[:])
            bk_sb = consts.tile([P, NKT], F32)
            nc.sync.dma_start(bk_sb[:], bk[:])
            bv_sb = consts.tile([1, D], FP16)
            nc.sync.dma_start(bv_sb[:], bv[:])
            bo_sb = consts.tile([1, D2], FP16)
            nc.sync.dma_start(bo_sb[:], bo[:])
            zb_sb = consts.tile([P, NJT], F32)
            nc.sync.dma_start(zb_sb[:], zb[:])
            ones_col = consts.tile([P, 1], FP16)
            nc.vector.memset(ones_col[:], 1.0)
            ones_row = consts.tile([1, P], FP16)
            nc.vector.memset(ones_row[:], 1.0)

            xT_sb = data.tile([P, NKT, L], FP16)
            nc.sync.dma_start(xT_sb[:], xT.ap().rearrange("(ko p) l -> p ko l", p=P))
            zT_sb = data.tile([P, NKT, L], FP16)
            nc.sync.dma_start(zT_sb[:], zT.ap().rearrange("(ko p) l -> p ko l", p=P))
            qT_sb = data.tile([P, NKT, L], FP16)
            kT_sb = data.tile([P, NKT, L], FP16)
            v_sb = data.tile([P, NJT, D], FP16)

            # ---- projections ----
            with tc.tile_pool(name="pjp", bufs=2, space="PSUM") as psum_pj:
                for w_sb, b_sb, src, dst in (
                    (wk_sb, bk_sb, zT_sb, kT_sb),
                    (wq_sb, bq_sb, xT_sb, qT_sb),
                ):
                    for c in range(NIC):
                        cs = slice(c * IC, (c + 1) * IC)
                        for h in range(NKT):
                            ps = psum_pj.tile([P, IC], F32, tag="pj")
                            for kt in range(NKT):
                                nc.tensor.matmul(
                                    ps[:],
                                    w_sb[:, kt, h * P : (h + 1) * P],
                                    src[:, kt, cs],
                                    start=(kt == 0),
                                    stop=(kt == NKT - 1),
                                )
                            nc.scalar.activation(
                                dst[:, h, cs],
                                ps[:],
                                mybir.ActivationFunctionType.Identity,
                                bias=b_sb[:, h : h + 1],
                            )
                for jt in range(NJT):
                    ps = psum_pj.tile([P, D], F32, tag="pjv")
                    for kt in range(NKT):
                        nc.tensor.matmul(
                            ps[:],
                            zT_sb[:, kt, jt * P : (jt + 1) * P],
                            wv_sb[:, kt, :],
                            start=(kt == 0),
                            stop=False,
                        )
                    nc.tensor.matmul(ps[:], ones_row[:], bv_sb[:], start=False, stop=True)
                    nc.vector.tensor_copy(v_sb[:, jt, :], ps[:])

            # ---- attention + FiLM ----
            with (
                tc.tile_pool(name="ps_s", bufs=2, space="PSUM") as psum_s,
                tc.tile_pool(name="ps_c0", bufs=1, space="PSUM") as psum_c0,
                tc.tile_pool(name="ps_c1", bufs=1, space="PSUM") as psum_c1,
                tc.tile_pool(name="ps_S", bufs=1, space="PSUM") as psum_S,
                tc.tile_pool(name="ps_rs", bufs=1, space="PSUM") as psum_rs,
                tc.tile_pool(name="ps_gb", bufs=2, space="PSUM") as psum_gb,
                tc.tile_pool(name="pe", bufs=6) as pool_e,
                tc.tile_pool(name="pw", bufs=2) as pool_w,
                tc.tile_pool(name="px", bufs=3) as pool_x,
            ):
                for c in range(NIC):
                    cs = slice(c * IC, (c + 1) * IC)
                    ctx0 = psum_c0.tile([P, IC], F32, tag="c0")
                    ctx1 = psum_c1.tile([P, IC], F32, tag="c1")
                    Sp = psum_S.tile([1, IC], F32, tag="S")
                    for jt in range(NJT):
                        sp = psum_s.tile([P, IC], F32, tag="s")
                        for h in range(NKT):
                            nc.tensor.matmul(
                                sp[:],
                                kT_sb[:, h, jt * P : (jt + 1) * P],
                                qT_sb[:, h, cs],
                                start=(h == 0),
                                stop=(h == NKT - 1),
                            )
                        e = pool_e.tile([P, IC], FP16, tag="E")
                        nc.scalar.activation(
                            e[:], sp[:], EXP, bias=zb_sb[:, jt : jt + 1], scale=1.0
                        )
                        nc.tensor.matmul(
                            ctx0[:],
                            v_sb[:, jt, 0:P],
                            e[:],
                            start=(jt == 0),
                            stop=(jt == NJT - 1),
                        )
                        nc.tensor.matmul(
                            ctx1[:],
                            v_sb[:, jt, P:D],
                            e[:],
                            start=(jt == 0),
                            stop=(jt == NJT - 1),
                        )
                        nc.tensor.matmul(
                            Sp[:],
                            ones_col[:],
                            e[:],
                            start=(jt == 0),
                            stop=(jt == NJT - 1),
                        )
                    S_sb = pool_w.tile([1, IC], FP16, tag="S_sb")
                    nc.vector.tensor_copy(S_sb[:], Sp[:])
                    rs = psum_rs.tile([P, NIT], F32, tag="rs")
                    for it in range(NIT):
                        nc.tensor.matmul(
                            rs[:, it : it + 1],
                            S_sb[:, it * P : (it + 1) * P],
                            ones_row[:, 0:1],
                            start=True,
                            stop=True,
                        )
                    recip = pool_w.tile([P, NIT], F32, tag="recip")
                    nc.vector.reciprocal(recip[:], rs[:])
                    c0_sb = pool_w.tile([P, IC], FP16, tag="c0sb")
                    c1_sb = pool_w.tile([P, IC], FP16, tag="c1sb")
                    nc.scalar.copy(c0_sb[:], ctx0[:])
                    nc.scalar.copy(c1_sb[:], ctx1[:])
                    for it in range(NIT):
                        g = c * NIT + it
                        gb = psum_gb.tile([P, D2], F32, tag="gb")
                        nc.tensor.matmul(
                            gb[:],
                            c0_sb[:, it * P : (it + 1) * P],
                            wo_sb[:, 0, :],
                            start=True,
                            stop=False,
                        )
                        nc.tensor.matmul(
                            gb[:],
                            c1_sb[:, it * P : (it + 1) * P],
                            wo_sb[:, 1, :],
                            start=False,
                            stop=False,
                        )
                        nc.tensor.matmul(
                            gb[:],
                            S_sb[:, it * P : (it + 1) * P],
                            bo_sb[:],
                            start=False,
                            stop=True,
                        )
                        xt = pool_x.tile([P, D], F32, tag="x")
                        nc.sync.dma_start(xt[:], xf[g * P : (g + 1) * P, :])
                        t = pool_x.tile([P, D], F32, tag="t")
                        nc.vector.tensor_tensor(t[:], gb[:, 0:D], xt[:], MUL)
                        nc.vector.tensor_tensor(t[:], t[:], gb[:, D:D2], ADD)
                        o = pool_x.tile([P, D], F32, tag="o")
                        nc.vector.tensor_tensor(
                            o[:],
                            t[:],
                            recip[:, it : it + 1].to_broadcast((P, D)),
                            MUL,
                        )
                        nc.sync.dma_start(out[g * P : (g + 1) * P, :], o[:])
    return nc


def get_nc():
    if "nc" not in _cache:
        _cache["nc"] = _build_nc()
    return _cache["nc"]


def make_in_maps(x, z, z_mask, Wq, bq, Wk, bk, Wv, bv, Wo, bo):
    x = np.asarray(x, np.float32)
    z = np.asarray(z, np.float32)
    z_mask = np.asarray(z_mask)
    scale = np.float32(1.0 / np.sqrt(D))
    wq_h = (np.asarray(Wq, np.float32) * scale).astype(np.float16)
    wk_h = np.asarray(Wk, np.float32).astype(np.float16)
    wv_h = np.asarray(Wv, np.float32).astype(np.float16)
    wo_h = np.asarray(Wo, np.float32).astype(np.float16)
    bq_r = np.ascontiguousarray(
        (np.asarray(bq, np.float32) * scale).reshape(NKT, P).T
    )
    bk_r = np.ascontiguousarray(np.asarray(bk, np.float32).reshape(NKT, P).T)
    bv_h = np.asarray(bv, np.float32).astype(np.float16).reshape(1, D)
    bo_h = np.asarray(bo, np.float32).astype(np.float16).reshape(1, D2)
    in_maps = []
    for b in range(B):
        zb_b = np.ascontiguousarray(
            np.where(z_mask[b], 0.0, NEG).astype(np.float32).reshape(NJT, P).T
        )
        in_maps.append(
            {
                "xT": np.ascontiguousarray(x[b].T).astype(np.float16),
                "zT": np.ascontiguousarray(z[b].T).astype(np.float16),
                "xf": np.ascontiguousarray(x[b]),
                "wq": wq_h,
                "wk": wk_h,
                "wv": wv_h,
                "wo": wo_h,
                "bq": bq_r,
                "bk": bk_r,
                "bv": bv_h,
                "bo": bo_h,
                "zb": zb_b,
            }
        )
    return in_maps


def kernel(x, z, x_mask, z_mask, Wq, bq, Wk, bk, Wv, bv, Wo, bo):
    global LAST_RESULTS
    nc = get_nc()
    in_maps = make_in_maps(x, z, z_mask, Wq, bq, Wk, bk, Wv, bv, Wo, bo)
    res = run_bass_kernel_spmd(nc, in_maps, list(range(NCORES)), trace=TRACE)
    LAST_RESULTS = res
    return np.stack([res.results[c]["out"] for c in range(NCORES)], axis=0)
